# revision 1
# baseline (speedup 1.0000x reference)
"""LocalWindowAttention (block-causal) Trainium2 kernel, 8 NeuronCores.

Sharding: tensor-parallel over heads. Core c owns head-columns
[c*128, (c+1)*128) of the D=1024 hidden dim (2 heads x head_dim 64):
  - computes Q/K/V projections for its head slice (transposed layout),
  - block-causal attention for its 2 heads,
  - partial output projection with its 128 rows of Wo,
  - chunked ReduceScatter(add) sums partials; core c keeps rows
    [c*128,(c+1)*128) of final^T [1024, 2048]. Host reassembles.

All big matmuls run in float32r (fp32 with ~13-bit mantissa rounding on
the PE read path): 1 cycle/row for free dim >= 256 -- 4x faster than
plain fp32, ~32x more precise than bf16.

Attention runs in S^T layout (keys on partitions, queries free):
S^T tile = K_chunk @ Q^T. No max-subtraction needed (scores bounded),
and the softmax denominator comes free from a ones-column appended to
the V operand of the attn@V matmul (output row 64 = sum_k exp(s)).
The two heads are interleaved so the exp (ACT engine) of one head
hides behind the other head's matmuls, keeping the PE dense and the
HAM clock un-throttled. Query chunks are processed in descending
visibility order so each chunk's partial output projection and its
ReduceScatter slice overlap the remaining attention compute.
"""

import numpy as np

import concourse.bacc as bacc
import concourse.tile as tile
from concourse import mybir
from concourse.bass_utils import run_bass_kernel_spmd
from concourse.masks import make_identity

B, T, D = 1, 2048, 1024
H, HD, W = 16, 64, 128
N_CORES = 8
HS = D // N_CORES        # 128 head-columns per core (2 heads)
HPC = H // N_CORES       # heads per core
QW = 512                 # query-chunk width (free dim of S^T tiles)
NQ = T // QW             # 4 query chunks
NK = T // W              # 16 key chunks of 128
ND = D // 128            # 8 contraction chunks over D
SCALE = HD ** -0.5

F32 = mybir.dt.float32
F32R = mybir.dt.float32r
BF16 = mybir.dt.bfloat16
Exp = mybir.ActivationFunctionType.Exp

_compiled = {}


def _build():
    nc = bacc.Bacc("TRN2", target_bir_lowering=False, debug=False,
                   num_devices=N_CORES)
    xT_ap = nc.dram_tensor("xT", [D, T], F32R, kind="ExternalInput").ap()
    wq_ap = nc.dram_tensor("wq", [D, HS], F32R, kind="ExternalInput").ap()
    wk_ap = nc.dram_tensor("wk", [D, HS], F32R, kind="ExternalInput").ap()
    wv_ap = nc.dram_tensor("wv", [D, HS], F32R, kind="ExternalInput").ap()
    wo_ap = nc.dram_tensor("wo", [D, HS], F32R, kind="ExternalInput").ap()
    y_ap = nc.dram_tensor("y", [HS, T], F32, kind="ExternalOutput").ap()

    with tile.TileContext(nc) as tc:
        _body(tc, xT_ap, wq_ap, wk_ap, wv_ap, wo_ap, y_ap)
    nc.compile()
    return nc


def _body(tc, xT_ap, wq_ap, wk_ap, wv_ap, wo_ap, y_ap):
    nc = tc.nc
    from contextlib import ExitStack
    with ExitStack() as ctx:
        singles = ctx.enter_context(tc.tile_pool(name="singles", bufs=1))
        work = ctx.enter_context(tc.tile_pool(name="work", bufs=4))
        es_pool = ctx.enter_context(tc.tile_pool(name="es_pool", bufs=8))
        dram = ctx.enter_context(tc.tile_pool(name="dram", bufs=1, space="DRAM"))

        # ---- load inputs (weights first: small, unblock first matmuls) ----
        wq = singles.tile([128, ND, HS], F32R, tag="wq")
        wk = singles.tile([128, ND, HS], F32R, tag="wk")
        wv = singles.tile([128, ND, HS], F32R, tag="wv")
        nc.sync.dma_start(out=wq[:], in_=wq_ap.rearrange("(c p) m -> p c m", p=128))
        nc.sync.dma_start(out=wk[:], in_=wk_ap.rearrange("(c p) m -> p c m", p=128))
        nc.sync.dma_start(out=wv[:], in_=wv_ap.rearrange("(c p) m -> p c m", p=128))
        # x chunks 0-3 stream on the sync queue right behind the weights;
        # chunks 4-7 + wo go on the scalar engine's queue in parallel (ACT
        # is idle during the load phase)
        xts = []
        for d in range(ND):
            xt = singles.tile([128, T], F32R, tag=f"x{d}", name=f"xt{d}")
            eng = nc.sync if d < 4 else nc.scalar
            eng.dma_start(out=xt[:], in_=xT_ap[d * 128:(d + 1) * 128, :])
            xts.append(xt)
        wo = singles.tile([128, ND, HS], F32R, tag="wo")
        nc.scalar.dma_start(out=wo[:], in_=wo_ap.rearrange("(c p) m -> p c m", p=128))

        ident_f32 = singles.tile([128, 128], F32, tag="ident_f32")
        make_identity(nc, ident_f32)
        ident = singles.tile([128, 128], F32R, tag="ident")
        nc.vector.tensor_copy(ident[:], ident_f32[:])

        qT = singles.tile([128, T], F32R, tag="qT")
        kT = singles.tile([128, T], F32R, tag="kT")
        vT = singles.tile([128, T], F32R, tag="vT")
        # V in natural layout per head: [key 128, NK chunks, HD + ones col]
        vn = [singles.tile([128, NK, HD + 1], F32R, tag=f"vn{h}", name=f"vn{h}")
              for h in range(HPC)]
        outT = singles.tile([128, T], F32R, tag="outT")

        # ---- Q/K/V projections (transposed layout) ------------------------
        with tc.tile_pool(name="pp", bufs=2, space="PSUM") as pp:
            for t in range(NQ):
                ps_q = pp.tile([128, QW], F32, tag="q")
                ps_k = pp.tile([128, QW], F32, tag="k")
                ps_v = pp.tile([128, QW], F32, tag="v")
                cols = slice(t * QW, (t + 1) * QW)
                for d in range(ND):
                    f = (d == 0)
                    l = (d == ND - 1)
                    nc.tensor.matmul(ps_q[:], wq[:, d, :], xts[d][:, cols], start=f, stop=l)
                    nc.tensor.matmul(ps_k[:], wk[:, d, :], xts[d][:, cols], start=f, stop=l)
                    nc.tensor.matmul(ps_v[:], wv[:, d, :], xts[d][:, cols], start=f, stop=l)
                nc.vector.tensor_copy(qT[:, cols], ps_q[:])
                nc.vector.tensor_copy(kT[:, cols], ps_k[:])
                nc.vector.tensor_copy(vT[:, cols], ps_v[:])

        # ---- transpose V to natural layout, append ones column ------------
        ones = singles.tile([128, 1], F32, tag="ones")
        nc.vector.memset(ones[:], 1.0)
        for h in range(HPC):
            nc.vector.tensor_copy(vn[h][:, :, HD:],
                                  ones[:].unsqueeze(1).to_broadcast([128, NK, 1]))
        with tc.tile_pool(name="pt", bufs=3, space="PSUM") as pt:
            for tk in range(NK):
                ps_t = pt.tile([128, 128], F32R, tag="t")
                nc.tensor.transpose(
                    ps_t[:], vT[:, tk * W:(tk + 1) * W], ident[:])
                for h in range(HPC):
                    nc.vector.tensor_copy(vn[h][:, tk, :HD],
                                          ps_t[:, h * HD:(h + 1) * HD])

        # ---- attention + output projection, query chunks descending -------
        ag_in = [dram.tile([HS, 2 * QW], F32R, name=f"ag_in{j}")
                 for j in range(NQ // 2)]
        ag_out = [dram.tile([N_CORES, HS, 2 * QW], F32R, addr_space="Shared",
                            name=f"ag_out{j}") for j in range(NQ // 2)]

        with tc.tile_pool(name="pa", bufs=2, space="PSUM") as pa, \
             tc.tile_pool(name="po", bufs=2, space="PSUM") as po:
            for t in range(NQ):
                cols = slice(t * QW, (t + 1) * QW)
                n_tk = 4 * t + 4
                ps_o = [po.tile([HD + 1, QW], F32, tag=f"o{h}", name=f"ps_o{h}")
                        for h in range(HPC)]

                def s_exp(tk):
                    qs = max(0, (tk - 4 * t) * W)  # masked cols before qs
                    # both heads' scores go into one double-wide PSUM tile so
                    # a single ACT instruction computes both exps (lower ACT
                    # overhead keeps the scalar engine ahead of the PE)
                    ps_s = pa.tile([128, 2 * QW], F32, tag="s", name="ps_s")
                    for h in range(HPC):
                        hrows = slice(h * HD, (h + 1) * HD)
                        nc.tensor.matmul(
                            ps_s[:, h * QW + qs:(h + 1) * QW],
                            kT[hrows, tk * W:(tk + 1) * W],
                            qT[hrows, t * QW + qs:(t + 1) * QW],
                            start=True, stop=True)
                    e = es_pool.tile([128, 2 * QW], F32R, tag="es", name="es")
                    nc.scalar.activation(out=e[:, qs:], in_=ps_s[:, qs:],
                                         func=Exp, scale=SCALE)
                    return e

                def av(tk, e):
                    qs = max(0, (tk - 4 * t) * W)
                    for h in range(HPC):
                        nc.tensor.matmul(ps_o[h][:, qs:], vn[h][:, tk, :],
                                         e[:, h * QW + qs:(h + 1) * QW],
                                         start=(tk == 0), stop=(tk == n_tk - 1))

                # software pipeline: scores/exp run two key-chunks ahead of
                # the attn@V accumulation so the PE always has independent
                # matmuls queued while the ACT works (n_tk >= 4 always)
                pend = [s_exp(0), s_exp(1)]
                for tk in range(2, n_tk):
                    pend.append(s_exp(tk))
                    av(tk - 2, pend.pop(0))
                av(n_tk - 2, pend.pop(0))
                av(n_tk - 1, pend.pop(0))

                # normalize: rows 0..63 / row 64
                for h in range(HPC):
                    hrows = slice(h * HD, (h + 1) * HD)
                    r_sb = work.tile([1, QW], F32, tag="rsb")
                    nc.vector.tensor_copy(r_sb[:], ps_o[h][HD:, :])
                    rec1 = work.tile([1, QW], F32, tag="rec1")
                    nc.vector.reciprocal(out=rec1[:], in_=r_sb[:])
                    bc = work.tile([HD, QW], F32, tag="bc")
                    nc.gpsimd.partition_broadcast(bc[:], rec1[:])
                    nc.vector.tensor_mul(outT[hrows, cols], ps_o[h][:HD, :], bc[:])

                # kick off a pair AllGather after each odd chunk; consumers
                # are emitted after ALL attention so the PE never stalls on it
                if t % 2 == 1:
                    j = t // 2
                    pcols = slice((t - 1) * QW, (t + 1) * QW)
                    nc.sync.dma_start(out=ag_in[j][:], in_=outT[:, pcols])
                    nc.gpsimd.collective_compute(
                        "AllGather", mybir.AluOpType.bypass,
                        replica_groups=[list(range(N_CORES))],
                        ins=[ag_in[j].opt()], outs=[ag_out[j].opt()])

        # apply the full Wo to the gathered activations: for our 128 output
        # columns, final^T[c-slice, cols] = sum_c Wo[c-block, slice]^T @ outT_c
        with tc.tile_pool(name="pf", bufs=2, space="PSUM") as pf, \
             tc.tile_pool(name="gt_pool", bufs=3) as gt_pool:
            for j in range(NQ // 2):
                ps_y = [pf.tile([128, QW], F32, tag=f"y{i}", name=f"ps_y{i}")
                        for i in range(2)]
                for c in range(N_CORES):
                    g = gt_pool.tile([128, 2 * QW], F32R, tag="g", name="g")
                    nc.sync.dma_start(out=g[:], in_=ag_out[j][c])
                    for i in range(2):
                        nc.tensor.matmul(ps_y[i][:], wo[:, c, :],
                                         g[:, i * QW:(i + 1) * QW],
                                         start=(c == 0), stop=(c == N_CORES - 1))
                for i in range(2):
                    t = 2 * j + i
                    cols = slice(t * QW, (t + 1) * QW)
                    cy = work.tile([128, QW], F32, tag="cy")
                    nc.vector.tensor_copy(cy[:], ps_y[i][:])
                    nc.sync.dma_start(out=y_ap[:, cols], in_=cy[:])


def kernel(x, Wq, Wk, Wv, Wo):
    if "nc" not in _compiled:
        _compiled["nc"] = _build()
    nc = _compiled["nc"]

    xT = np.ascontiguousarray(x.reshape(T, D).T.astype(np.float32))
    in_maps = []
    for c in range(N_CORES):
        hs = slice(c * HS, (c + 1) * HS)
        in_maps.append({
            "xT": xT,
            "wq": np.ascontiguousarray(Wq[:, hs].astype(np.float32)),
            "wk": np.ascontiguousarray(Wk[:, hs].astype(np.float32)),
            "wv": np.ascontiguousarray(Wv[:, hs].astype(np.float32)),
            "wo": np.ascontiguousarray(Wo[:, hs].astype(np.float32)),
        })
    res = run_bass_kernel_spmd(nc, in_maps, list(range(N_CORES)))
    finalT = np.concatenate([res.results[c]["y"] for c in range(N_CORES)], axis=0)
    return np.ascontiguousarray(finalT.T).reshape(B, T, D)



# revision 3
# speedup vs baseline: 1.1397x; 1.1397x over previous
"""LocalWindowAttention (block-causal) Trainium2 kernel, 8 NeuronCores.

Sharding: tensor-parallel over heads. Core c owns head-columns
[c*128, (c+1)*128) of the D=1024 hidden dim (2 heads x head_dim 64):
  - computes Q/K/V projections for its head slice (transposed layout),
  - block-causal attention for its 2 heads,
  - AllGathers its normalized attention outputs per query chunk,
  - applies the full Wo to the gathered activations for its 128 output
    rows of final^T [1024, 2048]. Host reassembles.

v2 changes vs baseline:
  - bf16 operands everywhere on-chip (PSUM accumulation stays f32):
    1 cycle/row at ANY free dim (no fp32r <256 penalty), half the DMA.
  - x is uploaded bf16 and streamed in d-chunk order on two queues;
    projections run outer-d so the first matmul fires as soon as x
    chunk 0 lands (PE starts ~2us instead of ~35us).
  - a tiny warmup AllGather at kernel start absorbs the cross-core
    launch skew and the CC engine's cold-start (~11us) so the real
    AllGathers run warm.
  - one AllGather per 512-col query chunk, triggered immediately after
    that chunk's normalize; the output projection for chunks 0-2 is
    emitted after all attention, covering the latency of chunk 3's
    AllGather.
  - softmax normalize: denominator reciprocal via exp(-ln d) on the ACT
    engine (ln/exp share one LUT set, no table swaps) instead of the
    6 cycle/element DVE reciprocal; then one Pool partition_broadcast
    and one DVE multiply per head.
"""

import numpy as np
import ml_dtypes

import concourse.bacc as bacc
import concourse.tile as tile
from concourse import mybir
from concourse.bass_utils import run_bass_kernel_spmd
from concourse.masks import make_identity

B, T, D = 1, 2048, 1024
H, HD, W = 16, 64, 128
N_CORES = 8
HS = D // N_CORES        # 128 head-columns per core (2 heads)
HPC = H // N_CORES       # heads per core
QW = 512                 # query-chunk width (free dim of S^T tiles)
NQ = T // QW             # 4 query chunks
NK = T // W              # 16 key chunks of 128
ND = D // 128            # 8 contraction chunks over D
SCALE = HD ** -0.5

F32 = mybir.dt.float32
BF16 = mybir.dt.bfloat16
Exp = mybir.ActivationFunctionType.Exp
Ln = mybir.ActivationFunctionType.Ln

_compiled = {}


def _build():
    nc = bacc.Bacc("TRN2", target_bir_lowering=False, debug=False,
                   num_devices=N_CORES)
    xT_ap = nc.dram_tensor("xT", [D, T], BF16, kind="ExternalInput").ap()
    wq_ap = nc.dram_tensor("wq", [D, HS], BF16, kind="ExternalInput").ap()
    wk_ap = nc.dram_tensor("wk", [D, HS], BF16, kind="ExternalInput").ap()
    wv_ap = nc.dram_tensor("wv", [D, HS], BF16, kind="ExternalInput").ap()
    wo_ap = nc.dram_tensor("wo", [D, HS], BF16, kind="ExternalInput").ap()
    y_ap = nc.dram_tensor("y", [HS, T], F32, kind="ExternalOutput").ap()

    with tile.TileContext(nc) as tc:
        _body(tc, xT_ap, wq_ap, wk_ap, wv_ap, wo_ap, y_ap)
    nc.compile()
    return nc


def _body(tc, xT_ap, wq_ap, wk_ap, wv_ap, wo_ap, y_ap):
    nc = tc.nc
    from contextlib import ExitStack
    with ExitStack() as ctx:
        singles = ctx.enter_context(tc.tile_pool(name="singles", bufs=1))
        work = ctx.enter_context(tc.tile_pool(name="work", bufs=4))
        es_pool = ctx.enter_context(tc.tile_pool(name="es_pool", bufs=8))
        dram = ctx.enter_context(tc.tile_pool(name="dram", bufs=1, space="DRAM"))

        # ---- identity (for V transposes) then warmup collective ----------
        ident_f32 = singles.tile([128, 128], F32, tag="ident_f32")
        make_identity(nc, ident_f32)
        ident = singles.tile([128, 128], BF16, tag="ident")
        nc.vector.tensor_copy(ident[:], ident_f32[:])

        # tiny AllGather: absorbs cross-core launch skew + CC cold start so
        # the per-chunk AllGathers below run warm and aligned
        warm_in = dram.tile([128, 8], F32, name="warm_in")
        warm_out = dram.tile([N_CORES, 128, 8], F32, addr_space="Shared",
                             name="warm_out")
        nc.gpsimd.collective_compute(
            "AllGather", mybir.AluOpType.bypass,
            replica_groups=[list(range(N_CORES))],
            ins=[warm_in.opt()], outs=[warm_out.opt()])

        # ---- load inputs (weights first; x in d-order on two queues) -----
        wq = singles.tile([128, ND, HS], BF16, tag="wq")
        wk = singles.tile([128, ND, HS], BF16, tag="wk")
        wv = singles.tile([128, ND, HS], BF16, tag="wv")
        wo = singles.tile([128, ND, HS], BF16, tag="wo")
        nc.sync.dma_start(out=wq[:], in_=wq_ap.rearrange("(c p) m -> p c m", p=128))
        nc.sync.dma_start(out=wk[:], in_=wk_ap.rearrange("(c p) m -> p c m", p=128))
        nc.scalar.dma_start(out=wv[:], in_=wv_ap.rearrange("(c p) m -> p c m", p=128))
        xts = []
        for d in range(ND):
            xt = singles.tile([128, T], BF16, tag=f"x{d}", name=f"xt{d}")
            eng = nc.sync if d % 2 == 0 else nc.scalar
            eng.dma_start(out=xt[:], in_=xT_ap[d * 128:(d + 1) * 128, :])
            xts.append(xt)
        nc.scalar.dma_start(out=wo[:], in_=wo_ap.rearrange("(c p) m -> p c m", p=128))

        qT = singles.tile([128, T], BF16, tag="qT")
        kT = singles.tile([128, T], BF16, tag="kT")
        vT = singles.tile([128, T], BF16, tag="vT")
        # V in natural layout per head: [key 128, NK chunks, HD + ones col]
        vn = [singles.tile([128, NK, HD + 1], BF16, tag=f"vn{h}", name=f"vn{h}")
              for h in range(HPC)]
        outT = singles.tile([128, T], BF16, tag="outT")

        ones = singles.tile([128, 1], F32, tag="ones")
        nc.vector.memset(ones[:], 1.0)
        for h in range(HPC):
            nc.vector.tensor_copy(vn[h][:, :, HD:],
                                  ones[:].unsqueeze(1).to_broadcast([128, NK, 1]))

        # ---- Q/K/V projections: two half-T passes, outer-d streaming -----
        # first matmul only needs wq + x chunk 0; the d-loop then consumes
        # x chunks as they stream in.
        with tc.tile_pool(name="pp", bufs=1, space="PSUM") as pp, \
             tc.tile_pool(name="pt", bufs=2, space="PSUM") as pt:
            for half in range(2):
                ps_q = pp.tile([128, 2, QW], F32, tag="q", name="ps_q")
                ps_k = pp.tile([128, 2, QW], F32, tag="k", name="ps_k")
                ps_v = pp.tile([128, 2, QW], F32, tag="v", name="ps_v")
                for d in range(ND):
                    f = (d == 0)
                    l = (d == ND - 1)
                    for ps, w in ((ps_q, wq), (ps_k, wk), (ps_v, wv)):
                        for sub in range(2):
                            cs = slice((2 * half + sub) * QW,
                                       (2 * half + sub + 1) * QW)
                            nc.tensor.matmul(ps[:, sub, :], w[:, d, :],
                                             xts[d][:, cs], start=f, stop=l)
                cols = slice(half * 2 * QW, (half + 1) * 2 * QW)
                nc.vector.tensor_copy(qT[:, cols], ps_q[:])
                nc.vector.tensor_copy(kT[:, cols], ps_k[:])
                nc.vector.tensor_copy(vT[:, cols], ps_v[:])

            # ---- transpose V to natural layout (after both passes) -------
            for tk in range(NK):
                ps_t = pt.tile([128, W], BF16, tag="t", name="ps_t")
                nc.tensor.transpose(
                    ps_t[:], vT[:, tk * W:(tk + 1) * W], ident[:])
                for h in range(HPC):
                    nc.vector.tensor_copy(vn[h][:, tk, :HD],
                                          ps_t[:, h * HD:(h + 1) * HD])

        # ---- attention; one eager AllGather per query chunk --------------
        ag_in = [dram.tile([HS, QW], BF16, name=f"ag_in{t}")
                 for t in range(NQ)]
        ag_out = [dram.tile([N_CORES, HS, QW], BF16, addr_space="Shared",
                            name=f"ag_out{t}") for t in range(NQ)]

        with tc.tile_pool(name="pa", bufs=2, space="PSUM") as pa, \
             tc.tile_pool(name="po", bufs=2, space="PSUM") as po:
            for t in range(NQ):
                cols = slice(t * QW, (t + 1) * QW)
                n_tk = 4 * t + 4
                # both heads' outputs in one tile; row 64 = denominators
                ps_o = po.tile([HD + 1, 2, QW], F32, tag="o", name="ps_o")

                def s_exp(tk):
                    qs = max(0, (tk - 4 * t) * W)  # masked cols before qs
                    # both heads' scores in one double-wide PSUM tile so a
                    # single ACT instruction computes both exps
                    ps_s = pa.tile([128, 2 * QW], F32, tag="s", name="ps_s")
                    for h in range(HPC):
                        hrows = slice(h * HD, (h + 1) * HD)
                        nc.tensor.matmul(
                            ps_s[:, h * QW + qs:(h + 1) * QW],
                            kT[hrows, tk * W:(tk + 1) * W],
                            qT[hrows, t * QW + qs:(t + 1) * QW],
                            start=True, stop=True)
                    e = es_pool.tile([128, 2 * QW], BF16, tag="es", name="es")
                    nc.scalar.activation(out=e[:, qs:], in_=ps_s[:, qs:],
                                         func=Exp, scale=SCALE)
                    return e

                def av(tk, e):
                    qs = max(0, (tk - 4 * t) * W)
                    for h in range(HPC):
                        nc.tensor.matmul(ps_o[:, h, qs:], vn[h][:, tk, :],
                                         e[:, h * QW + qs:(h + 1) * QW],
                                         start=(tk == 0), stop=(tk == n_tk - 1))

                # software pipeline: scores/exp run two key-chunks ahead of
                # the attn@V accumulation (n_tk >= 4 always)
                pend = [s_exp(0), s_exp(1)]
                for tk in range(2, n_tk):
                    pend.append(s_exp(tk))
                    av(tk - 2, pend.pop(0))
                av(n_tk - 2, pend.pop(0))
                av(n_tk - 1, pend.pop(0))

                # normalize: rows 0..63 of each head / row 64.
                # 1/d computed as exp(-ln d) on ACT (ln+exp share a LUT set).
                lnd = work.tile([1, 2, QW], F32, tag="lnd")
                nc.scalar.activation(out=lnd[:], in_=ps_o[HD:, :, :], func=Ln)
                rec = work.tile([1, 2, QW], F32, tag="rec")
                nc.scalar.activation(out=rec[:], in_=lnd[:], func=Exp,
                                     scale=-1.0)
                bc = work.tile([HD, 2, QW], F32, tag="bc")
                nc.gpsimd.partition_broadcast(bc[:], rec[:])
                for h in range(HPC):
                    hrows = slice(h * HD, (h + 1) * HD)
                    nc.vector.tensor_mul(outT[hrows, cols],
                                         ps_o[:HD, h, :], bc[:, h, :])

                # eager AllGather for this chunk (CC engine runs them in
                # order; consumers are emitted after all attention)
                nc.gpsimd.dma_start(out=ag_in[t][:], in_=outT[:, cols])
                nc.gpsimd.collective_compute(
                    "AllGather", mybir.AluOpType.bypass,
                    replica_groups=[list(range(N_CORES))],
                    ins=[ag_in[t].opt()], outs=[ag_out[t].opt()])

        # ---- output projection: full Wo on gathered activations ----------
        # final^T[c-slice, cols] = sum_c Wo[c-block, slice]^T @ outT_c
        with tc.tile_pool(name="py", bufs=2, space="PSUM") as py, \
             tc.tile_pool(name="gt_pool", bufs=3) as gt_pool:
            for t in range(NQ):
                cols = slice(t * QW, (t + 1) * QW)
                ps_y = py.tile([128, QW], F32, tag="y", name="ps_y")
                for c in range(N_CORES):
                    g = gt_pool.tile([HS, QW], BF16, tag="g", name="g")
                    nc.sync.dma_start(out=g[:], in_=ag_out[t][c])
                    nc.tensor.matmul(ps_y[:], wo[:, c, :], g[:],
                                     start=(c == 0), stop=(c == N_CORES - 1))
                cy = work.tile([128, QW], F32, tag="cy")
                nc.vector.tensor_copy(cy[:], ps_y[:])
                nc.sync.dma_start(out=y_ap[:, cols], in_=cy[:])


def make_in_maps(x, Wq, Wk, Wv, Wo):
    bf = ml_dtypes.bfloat16
    xT = np.ascontiguousarray(np.asarray(x).reshape(T, D).T).astype(bf)
    in_maps = []
    for c in range(N_CORES):
        hs = slice(c * HS, (c + 1) * HS)
        in_maps.append({
            "xT": xT,
            "wq": np.ascontiguousarray(np.asarray(Wq)[:, hs]).astype(bf),
            "wk": np.ascontiguousarray(np.asarray(Wk)[:, hs]).astype(bf),
            "wv": np.ascontiguousarray(np.asarray(Wv)[:, hs]).astype(bf),
            "wo": np.ascontiguousarray(np.asarray(Wo)[:, hs]).astype(bf),
        })
    return in_maps


def kernel(x, Wq, Wk, Wv, Wo):
    if "nc" not in _compiled:
        _compiled["nc"] = _build()
    nc = _compiled["nc"]

    in_maps = make_in_maps(x, Wq, Wk, Wv, Wo)
    res = run_bass_kernel_spmd(nc, in_maps, list(range(N_CORES)))
    finalT = np.concatenate([res.results[c]["y"] for c in range(N_CORES)], axis=0)
    return np.ascontiguousarray(finalT.T).reshape(B, T, D)


# revision 9
# speedup vs baseline: 1.4815x; 1.2999x over previous
"""LocalWindowAttention (block-causal) Trainium2 kernel, 8 NeuronCores.

Sharding: tensor-parallel over heads. Core c owns head-columns
[c*128, (c+1)*128) of the D=1024 hidden dim (2 heads x head_dim 64):
  - computes Q/K/V projections for its head slice (transposed layout),
  - block-causal attention for its 2 heads,
  - AllGathers its normalized attention outputs (two chunk-pair AGs),
  - applies the full Wo to the gathered activations for its 128 output
    rows of final^T [1024, 2048]. Host reassembles.

v3:
  - bf16 operands on-chip (PSUM stays f32): 1 cycle/row at any free
    dim, half the DMA of f32.
  - x DMAs are emitted interleaved with the first projection pass's
    matmuls, round-robin over four queues, so each matmul's DMA wait
    only covers chunks emitted before it -> PE starts ~2.5us after the
    queues open instead of after the full x load.
  - tiny warmup AllGather absorbs the CC engine's ~45us cold start.
  - attention runs as one flat (chunk, keyblock) stream with the
    score/exp pipeline carried across chunk boundaries (no drain
    stalls); normalize is emitted inline after each chunk's last AV.
  - two AllGathers (chunks {0,1} after c1, {2,3} after c3); the
    {0,1} output projection covers the second AG's latency.
"""

import numpy as np
import ml_dtypes

import concourse.bacc as bacc
import concourse.tile as tile
from concourse import mybir
from concourse.bass_utils import run_bass_kernel_spmd
from concourse.masks import make_identity

B, T, D = 1, 2048, 1024
H, HD, W = 16, 64, 128
N_CORES = 8
HS = D // N_CORES        # 128 head-columns per core (2 heads)
HPC = H // N_CORES       # heads per core
QW = 512                 # query-chunk width (free dim of S^T tiles)
NQ = T // QW             # 4 query chunks
NK = T // W              # 16 key chunks of 128
ND = D // 128            # 8 contraction chunks over D
SCALE = HD ** -0.5

F32 = mybir.dt.float32
BF16 = mybir.dt.bfloat16
Exp = mybir.ActivationFunctionType.Exp

_compiled = {}


def _build():
    nc = bacc.Bacc("TRN2", target_bir_lowering=False, debug=False,
                   num_devices=N_CORES)
    xT_ap = nc.dram_tensor("xT", [D, T], BF16, kind="ExternalInput").ap()
    wq_ap = nc.dram_tensor("wq", [D, HS], BF16, kind="ExternalInput").ap()
    wk_ap = nc.dram_tensor("wk", [D, HS], BF16, kind="ExternalInput").ap()
    wv_ap = nc.dram_tensor("wv", [D, HS], BF16, kind="ExternalInput").ap()
    wo_ap = nc.dram_tensor("wo", [D, HS], BF16, kind="ExternalInput").ap()
    y_ap = nc.dram_tensor("y", [HS, T], F32, kind="ExternalOutput").ap()

    with tile.TileContext(nc) as tc:
        _body(tc, xT_ap, wq_ap, wk_ap, wv_ap, wo_ap, y_ap)
    nc.compile()
    return nc


def _body(tc, xT_ap, wq_ap, wk_ap, wv_ap, wo_ap, y_ap):
    nc = tc.nc
    from contextlib import ExitStack
    with ExitStack() as ctx:
        singles = ctx.enter_context(tc.tile_pool(name="singles", bufs=1))
        work = ctx.enter_context(tc.tile_pool(name="work", bufs=4))
        es_pool = ctx.enter_context(tc.tile_pool(name="es_pool", bufs=8))
        dram = ctx.enter_context(tc.tile_pool(name="dram", bufs=1, space="DRAM"))

        # ---- identity (for V transposes) then warmup collective ----------
        ident_f32 = singles.tile([128, 128], F32, tag="ident_f32")
        make_identity(nc, ident_f32)
        ident = singles.tile([128, 128], BF16, tag="ident")
        nc.vector.tensor_copy(ident[:], ident_f32[:])

        warm_in = dram.tile([128, 8], F32, name="warm_in")
        warm_out = dram.tile([N_CORES, 128, 8], F32, addr_space="Shared",
                             name="warm_out")
        nc.gpsimd.collective_compute(
            "AllGather", mybir.AluOpType.bypass,
            replica_groups=[list(range(N_CORES))],
            ins=[warm_in.opt()], outs=[warm_out.opt()])

        # ---- weight DMAs, then x chunks interleaved with pass-A matmuls --
        wq = singles.tile([128, ND, HS], BF16, tag="wq")
        wk = singles.tile([128, ND, HS], BF16, tag="wk")
        wv = singles.tile([128, ND, HS], BF16, tag="wv")
        wo = singles.tile([128, ND, HS], BF16, tag="wo")
        nc.sync.dma_start(out=wq[:], in_=wq_ap.rearrange("(c p) m -> p c m", p=128))
        nc.scalar.dma_start(out=wk[:], in_=wk_ap.rearrange("(c p) m -> p c m", p=128))
        nc.gpsimd.dma_start(out=wv[:], in_=wv_ap.rearrange("(c p) m -> p c m", p=128))

        qT = singles.tile([128, T], BF16, tag="qT")
        kT = singles.tile([128, T], BF16, tag="kT")
        vT = singles.tile([128, T], BF16, tag="vT")
        vn = [singles.tile([128, NK, HD + 1], BF16, tag=f"vn{h}", name=f"vn{h}")
              for h in range(HPC)]
        outT = singles.tile([128, T], BF16, tag="outT")

        ones = singles.tile([128, 1], F32, tag="ones")
        nc.vector.memset(ones[:], 1.0)
        for h in range(HPC):
            nc.vector.tensor_copy(vn[h][:, :, HD:],
                                  ones[:].unsqueeze(1).to_broadcast([128, NK, 1]))

        qdma = [nc.sync, nc.scalar, nc.gpsimd]
        xts = []

        with tc.tile_pool(name="pp", bufs=1, space="PSUM") as pp, \
             tc.tile_pool(name="pt", bufs=2, space="PSUM") as pt:
            # pass A (cols 0..1023): emit each x-chunk DMA right before the
            # matmuls that consume it so the DMA waits stay minimal
            ps_q = pp.tile([128, 2, QW], F32, tag="q", name="ps_q")
            ps_k = pp.tile([128, 2, QW], F32, tag="k", name="ps_k")
            ps_v = pp.tile([128, 2, QW], F32, tag="v", name="ps_v")
            for d in range(ND):
                xt = singles.tile([128, T], BF16, tag=f"x{d}", name=f"xt{d}")
                qdma[d % 3].dma_start(out=xt[:], in_=xT_ap[d * 128:(d + 1) * 128, :])
                xts.append(xt)
                f = (d == 0)
                l = (d == ND - 1)
                for ps, w in ((ps_q, wq), (ps_k, wk), (ps_v, wv)):
                    for sub in range(2):
                        cs = slice(sub * QW, (sub + 1) * QW)
                        nc.tensor.matmul(ps[:, sub, :], w[:, d, :],
                                         xts[d][:, cs], start=f, stop=l)
            nc.gpsimd.dma_start(out=wo[:],
                                in_=wo_ap.rearrange("(c p) m -> p c m", p=128))
            nc.vector.tensor_copy(vT[:, 0:2 * QW], ps_v[:])
            nc.vector.tensor_copy(qT[:, 0:2 * QW], ps_q[:])
            nc.vector.tensor_copy(kT[:, 0:2 * QW], ps_k[:])
            # V transposes for blocks 0-7 (PE; waits only on the v copy)
            for tk in range(8):
                ps_t = pt.tile([128, W], BF16, tag="t", name="ps_t")
                nc.tensor.transpose(ps_t[:], vT[:, tk * W:(tk + 1) * W], ident[:])
                for h in range(HPC):
                    nc.vector.tensor_copy(vn[h][:, tk, :HD],
                                          ps_t[:, h * HD:(h + 1) * HD])

            # pass B (cols 1024..2047): all of x is resident by now
            ps_q = pp.tile([128, 2, QW], F32, tag="q", name="ps_qB")
            ps_k = pp.tile([128, 2, QW], F32, tag="k", name="ps_kB")
            ps_v = pp.tile([128, 2, QW], F32, tag="v", name="ps_vB")
            for d in range(ND):
                f = (d == 0)
                l = (d == ND - 1)
                for ps, w in ((ps_q, wq), (ps_k, wk), (ps_v, wv)):
                    for sub in range(2):
                        cs = slice(2 * QW + sub * QW, 2 * QW + (sub + 1) * QW)
                        nc.tensor.matmul(ps[:, sub, :], w[:, d, :],
                                         xts[d][:, cs], start=f, stop=l)
            nc.vector.tensor_copy(vT[:, 2 * QW:], ps_v[:])
            nc.vector.tensor_copy(qT[:, 2 * QW:], ps_q[:])
            nc.vector.tensor_copy(kT[:, 2 * QW:], ps_k[:])
            for tk in range(8, NK):
                ps_t = pt.tile([128, W], BF16, tag="t", name="ps_t")
                nc.tensor.transpose(ps_t[:], vT[:, tk * W:(tk + 1) * W], ident[:])
                for h in range(HPC):
                    nc.vector.tensor_copy(vn[h][:, tk, :HD],
                                          ps_t[:, h * HD:(h + 1) * HD])

        # ---- attention: flat (chunk, keyblock) stream, pipelined exps ----
        ag_in = [dram.tile([HS, 2 * QW], BF16, name=f"ag_in{j}")
                 for j in range(NQ // 2)]
        ag_out = [dram.tile([N_CORES, HS, 2 * QW], BF16, addr_space="Shared",
                            name=f"ag_out{j}") for j in range(NQ // 2)]

        with tc.tile_pool(name="pa", bufs=2, space="PSUM") as pa, \
             tc.tile_pool(name="po", bufs=2, space="PSUM") as po:
            ps_os = {}

            def s_exp(t, tk):
                qs = max(0, (tk - 4 * t) * W)
                ps_s = pa.tile([128, 2 * QW], F32, tag="s", name="ps_s")
                for h in range(HPC):
                    hrows = slice(h * HD, (h + 1) * HD)
                    nc.tensor.matmul(
                        ps_s[:, h * QW + qs:(h + 1) * QW],
                        kT[hrows, tk * W:(tk + 1) * W],
                        qT[hrows, t * QW + qs:(t + 1) * QW],
                        start=True, stop=True)
                e = es_pool.tile([128, 2 * QW], BF16, tag="es", name="es")
                nc.scalar.activation(out=e[:, qs:], in_=ps_s[:, qs:],
                                     func=Exp, scale=SCALE)
                return e

            def av(t, tk, e):
                qs = max(0, (tk - 4 * t) * W)
                n_tk = 4 * t + 4
                if tk == 0:
                    ps_os[t] = po.tile([HD + 1, 2, QW], F32, tag="o",
                                       name="ps_o")
                ps_o = ps_os[t]
                for h in range(HPC):
                    nc.tensor.matmul(ps_o[:, h, qs:], vn[h][:, tk, :],
                                     e[:, h * QW + qs:(h + 1) * QW],
                                     start=(tk == 0), stop=(tk == n_tk - 1))
                if tk == n_tk - 1:
                    _normalize(t, ps_o)

            def _normalize(t, ps_o):
                cols = slice(t * QW, (t + 1) * QW)
                bc = work.tile([HD, 2, QW], F32, tag="bc", name="bc")
                for h in range(HPC):
                    rec = work.tile([1, QW], F32, tag=f"rec{h}", name="rec")
                    nc.vector.reciprocal(out=rec[:], in_=ps_o[HD:, h, :])
                    nc.gpsimd.partition_broadcast(bc[:, h, :], rec[:])
                for h in range(HPC):
                    hrows = slice(h * HD, (h + 1) * HD)
                    nc.vector.tensor_mul(outT[hrows, cols],
                                         ps_o[:HD, h, :], bc[:, h, :])
                if t % 2 == 1:
                    j = t // 2
                    pcols = slice((t - 1) * QW, (t + 1) * QW)
                    nc.gpsimd.dma_start(out=ag_in[j][:], in_=outT[:, pcols])
                    nc.gpsimd.collective_compute(
                        "AllGather", mybir.AluOpType.bypass,
                        replica_groups=[list(range(N_CORES))],
                        ins=[ag_in[j].opt()], outs=[ag_out[j].opt()])

            # flat stream with the exp pipeline carried across chunks
            items = [(t, tk) for t in range(NQ) for tk in range(4 * t + 4)]
            pend = []
            for it in items:
                pend.append((it, s_exp(*it)))
                if len(pend) > 2:
                    (pt_, ptk), pe_ = pend.pop(0)
                    av(pt_, ptk, pe_)
            for (pt_, ptk), pe_ in pend:
                av(pt_, ptk, pe_)

        # ---- output projection on gathered activations -------------------
        # final^T[c-slice, cols] = sum_c Wo[c-block, slice]^T @ outT_c
        with tc.tile_pool(name="py", bufs=2, space="PSUM") as py, \
             tc.tile_pool(name="gt_pool", bufs=3) as gt_pool:
            for j in range(NQ // 2):
                ps_y = [py.tile([128, QW], F32, tag=f"y{i}", name=f"ps_y{i}")
                        for i in range(2)]
                for c in range(N_CORES):
                    g = gt_pool.tile([128, 2 * QW], BF16, tag="g", name="g")
                    nc.sync.dma_start(out=g[:], in_=ag_out[j][c])
                    for i in range(2):
                        nc.tensor.matmul(ps_y[i][:], wo[:, c, :],
                                         g[:, i * QW:(i + 1) * QW],
                                         start=(c == 0), stop=(c == N_CORES - 1))
                for i in range(2):
                    t = 2 * j + i
                    cols = slice(t * QW, (t + 1) * QW)
                    cy = work.tile([128, QW], F32, tag="cy")
                    nc.vector.tensor_copy(cy[:], ps_y[i][:])
                    nc.sync.dma_start(out=y_ap[:, cols], in_=cy[:])


def make_in_maps(x, Wq, Wk, Wv, Wo):
    bf = ml_dtypes.bfloat16
    xT = np.ascontiguousarray(np.asarray(x).reshape(T, D).T).astype(bf)
    in_maps = []
    for c in range(N_CORES):
        hs = slice(c * HS, (c + 1) * HS)
        in_maps.append({
            "xT": xT,
            "wq": np.ascontiguousarray(np.asarray(Wq)[:, hs]).astype(bf),
            "wk": np.ascontiguousarray(np.asarray(Wk)[:, hs]).astype(bf),
            "wv": np.ascontiguousarray(np.asarray(Wv)[:, hs]).astype(bf),
            "wo": np.ascontiguousarray(np.asarray(Wo)[:, hs]).astype(bf),
        })
    return in_maps


def kernel(x, Wq, Wk, Wv, Wo):
    if "nc" not in _compiled:
        _compiled["nc"] = _build()
    nc = _compiled["nc"]

    in_maps = make_in_maps(x, Wq, Wk, Wv, Wo)
    res = run_bass_kernel_spmd(nc, in_maps, list(range(N_CORES)))
    finalT = np.concatenate([res.results[c]["y"] for c in range(N_CORES)], axis=0)
    return np.ascontiguousarray(finalT.T).reshape(B, T, D)
